# revision 2
# baseline (speedup 1.0000x reference)
"""Trainium2 Bass kernel: 4096x4096 fp32 image, 16x16 valid cross-correlation + bias.

Phase-deinterleaved matmul conv (G=4): the host splits X's columns into
G=4 phases per 32-row strip,
    D[g*32+rho, n] = X[i0+rho, 4n+g],
so one banded-stationary matmul pass covers 4 kernel columns at once:
    psum[d*17+mu, J] += sum_p Sq[p, d*17+mu] * D[p, J+q]
    Sq[g*32+rho, d*17+mu] = w[rho-mu, 4q+g-d]   (when in range)
QP=5 passes replace the 16 column-shift passes of the direct banded
scheme — a ~3x cut in PE-streamed columns per output row. Outputs are
written as phase planes [68, 1024] per strip; the host re-interleaves.

All storage is bf16 (fp32 PSUM accumulation), halving HBM traffic; rel
err vs the fp32 reference ~3e-3, far under the 2e-2 gate. Strip loads
are column-split across the two HWDGE queues (sync/SP + scalar/Act);
outputs go on the gpsimd SWDGE queue. Output rows are sharded across
the 8 cores (512 rows each); weights and bias replicated.

Env (bench only): CONV_LOOP wraps the body in a hardware For_i loop.
"""
import os

import numpy as np

import concourse.mybir as mybir
import concourse.tile as tile
from concourse import bacc
from concourse.bass_utils import run_bass_kernel_spmd

H = 4096
W = 4096
KH = 16
KW = 16
OH = H - KH + 1  # 4081
OW = W - KW + 1  # 4081
NCORES = 8
RPC = 512  # output rows per core
N_T = 512  # psum tile free size (one fp32 PSUM bank)

G = 4  # column phases
R = 128 // G  # 32 strip rows
OUT_R = R - KH + 1  # 17 output rows per strip
QP = (G - 1 + KW - 1) // G + 1  # 5 matmul passes per strip
M = G * OUT_R  # 68 psum partitions
NW = W // G  # 1024 deinterleaved columns
NWP = NW + 8  # padded deint width
SPC = 31  # strips per core (31*17 = 527 >= 512)
NTILES = NW // N_T  # 2
CSPLIT = 516  # j=0 matmuls read cols 0..515; j=1 reads 512..1027

DT = mybir.dt.bfloat16
NP_BF16 = mybir.dt.np(mybir.dt.bfloat16)

_build_cache = {}


def _build():
    loop = int(os.environ.get("CONV_LOOP", "1"))
    if loop in _build_cache:
        return _build_cache[loop]

    nc = bacc.Bacc()
    x_d = nc.dram_tensor("Xd", [SPC, 128, NWP], DT, kind="ExternalInput")
    wq_d = nc.dram_tensor("wq", [128, QP, M], DT, kind="ExternalInput")
    bias_d = nc.dram_tensor("biasb", [128, 1], mybir.dt.float32, kind="ExternalInput")
    out_d = nc.dram_tensor("outp", [SPC, M, NW], DT, kind="ExternalOutput")

    with tile.TileContext(nc) as tc:
        with (
            tc.tile_pool(name="const", bufs=1) as cpool,
            tc.tile_pool(name="strip", bufs=4) as spool,
            tc.tile_pool(name="obuf", bufs=4) as opool,
            tc.tile_pool(name="psum", bufs=8, space="PSUM") as ppool,
        ):
            wq = cpool.tile([128, QP, M], DT)
            nc.gpsimd.dma_start(wq[:], wq_d[:])
            bias_sb = cpool.tile([128, 1], mybir.dt.float32)
            nc.gpsimd.dma_start(bias_sb[:], bias_d[:])

            def body():
                for s in range(SPC):
                    strip = spool.tile([128, NWP], DT, tag="strip")
                    nc.sync.dma_start(strip[:, 0:CSPLIT], x_d[s, :, 0:CSPLIT])
                    nc.scalar.dma_start(
                        strip[:, CSPLIT:NWP], x_d[s, :, CSPLIT:NWP]
                    )
                    ot = opool.tile([M, NW], DT, tag="ot")
                    for j in range(NTILES):
                        n0 = j * N_T
                        ps = ppool.tile([M, N_T], mybir.dt.float32, tag="ps")
                        for q in range(QP):
                            nc.tensor.matmul(
                                ps[:M, :N_T],
                                wq[:, q, :M],
                                strip[:, n0 + q : n0 + q + N_T],
                                start=(q == 0),
                                stop=(q == QP - 1),
                            )
                        nc.vector.tensor_scalar_add(
                            ot[:M, n0 : n0 + N_T], ps[:M, :N_T], bias_sb[:M]
                        )
                    nc.gpsimd.dma_start(out_d[s], ot[:M, :NW])

            if loop > 1:
                with tc.For_i(0, loop, 1):
                    body()
            else:
                body()
    nc.finalize()
    _build_cache[loop] = nc
    return nc


def _host_prep(X, weight, bias):
    Xb = X.astype(NP_BF16)
    pad_rows = RPC * (NCORES - 1) + OUT_R * (SPC - 1) + R  # 4126
    Xpad = np.zeros((pad_rows, W), dtype=NP_BF16)
    Xpad[:H] = Xb
    # Xdint[g][row, n] = Xpad[row, G*n+g]
    Xdint = np.ascontiguousarray(Xpad.reshape(pad_rows, NW, G).transpose(2, 0, 1))

    wb = weight.astype(NP_BF16)
    wq = np.zeros((128, QP, M), dtype=NP_BF16)
    for q in range(QP):
        for g in range(G):
            for d in range(G):
                c = G * q + g - d
                if not (0 <= c < KW):
                    continue
                for mu in range(OUT_R):
                    wq[g * R + mu : g * R + mu + KH, q, d * OUT_R + mu] = wb[:, c]
    biasb = np.full((128, 1), np.float32(bias[0]), dtype=np.float32)

    starts = (
        np.arange(NCORES)[:, None] * RPC + np.arange(SPC)[None, :] * OUT_R
    )  # [NCORES, SPC]
    rows = starts[:, :, None] + np.arange(R)[None, None, :]  # [NCORES, SPC, R]
    gathered = Xdint[:, rows, :]  # [G, NCORES, SPC, R, NW]

    in_maps = []
    for c in range(NCORES):
        Xd = np.zeros((SPC, 128, NWP), dtype=NP_BF16)
        Xd[:, :, :NW] = gathered[:, c].transpose(1, 0, 2, 3).reshape(SPC, 128, NW)
        in_maps.append({"Xd": Xd, "wq": wq, "biasb": biasb})
    return in_maps


def _host_post(results):
    rows = []
    for c in range(NCORES):
        outp = np.asarray(results[c]["outp"])  # [SPC, M, NW] bf16
        blk = (
            outp.reshape(SPC, G, OUT_R, NW)
            .transpose(0, 2, 3, 1)
            .reshape(SPC * OUT_R, W)
        )
        rows.append(blk[:RPC])
    full = np.concatenate(rows, axis=0)
    return np.ascontiguousarray(full[:OH, :OW]).astype(np.float32)


def kernel(X, weight, bias):
    X = np.asarray(X, dtype=np.float32)
    weight = np.asarray(weight, dtype=np.float32)
    bias = np.asarray(bias, dtype=np.float32)
    nc = _build()
    in_maps = _host_prep(X, weight, bias)
    res = run_bass_kernel_spmd(nc, in_maps, core_ids=list(range(NCORES)))
    return _host_post(res.results)


def _run(X, weight, bias, dt_name=None, trace=False):
    """Compatibility entry for test.py: returns (output, results)."""
    X = np.asarray(X, dtype=np.float32)
    weight = np.asarray(weight, dtype=np.float32)
    bias = np.asarray(bias, dtype=np.float32)
    nc = _build()
    in_maps = _host_prep(X, weight, bias)
    res = run_bass_kernel_spmd(
        nc, in_maps, core_ids=list(range(NCORES)), trace=trace
    )
    return _host_post(res.results), res


# revision 5
# speedup vs baseline: 1.1296x; 1.1296x over previous
"""Trainium2 Bass kernel: 4096x4096 fp32 image, 16x16 valid cross-correlation + bias.

Phase-deinterleaved matmul conv (G=4): the host splits X's columns into
G=4 phases per 32-row strip,
    D[g*32+rho, n] = X[i0+rho, 4n+g],
so one banded-stationary matmul pass covers 4 kernel columns at once:
    psum[d*17+mu, J] += sum_p Sq[p, d*17+mu] * D[p, J+q]
    Sq[g*32+rho, d*17+mu] = w[rho-mu, 4q+g-d]   (when in range)
QP=5 passes replace the 16 column-shift passes of the direct banded
scheme — a ~3x cut in PE-streamed columns per output row. Each tile's
5-pass accumulation is split 2+3 across two psum banks (accumulating
matmuls stream ~40% slower than resetting ones) and recombined on the
DVE. Outputs are written as phase planes [68, 1024] per strip; the
host re-interleaves.

All storage is bf16 (fp32 PSUM accumulation), halving HBM traffic; rel
err vs the fp32 reference ~3e-3, far under the 2e-2 gate. Strip loads
are column-split across the two HWDGE queues (sync/SP + scalar/Act);
outputs go on the gpsimd SWDGE queue. Output rows are sharded across
the 8 cores (512 rows each); weights and bias replicated.

Env (bench only): CONV_LOOP wraps the body in a hardware For_i loop.
"""
import os

import numpy as np

import concourse.mybir as mybir
import concourse.tile as tile
from concourse import bacc
from concourse.bass_utils import run_bass_kernel_spmd

H = 4096
W = 4096
KH = 16
KW = 16
OH = H - KH + 1  # 4081
OW = W - KW + 1  # 4081
NCORES = 8
RPC = 512  # output rows per core
N_T = 512  # psum tile free size (one fp32 PSUM bank)

G = 4  # column phases
R = 128 // G  # 32 strip rows
OUT_R = R - KH + 1  # 17 output rows per strip
QP = (G - 1 + KW - 1) // G + 1  # 5 matmul passes per strip
M = G * OUT_R  # 68 psum partitions
NW = W // G  # 1024 deinterleaved columns
NWP = NW + 8  # padded deint width
SPC = 31  # strips per core (31*17 = 527 >= 512)
NTILES = NW // N_T  # 2
CSPLIT = 516  # j=0 matmuls read cols 0..515; j=1 reads 512..1027

DT = mybir.dt.bfloat16
NP_BF16 = mybir.dt.np(mybir.dt.bfloat16)

_build_cache = {}


def _build():
    loop = int(os.environ.get("CONV_LOOP", "1"))
    if loop in _build_cache:
        return _build_cache[loop]

    nc = bacc.Bacc()
    x_d = nc.dram_tensor("Xd", [SPC, 128, NWP], DT, kind="ExternalInput")
    wq_d = nc.dram_tensor("wq", [128, QP, M], DT, kind="ExternalInput")
    bias_d = nc.dram_tensor("biasb", [128, 1], mybir.dt.float32, kind="ExternalInput")
    out_d = nc.dram_tensor("outp", [SPC, M, NW], DT, kind="ExternalOutput")

    with tile.TileContext(nc) as tc:
        with (
            tc.tile_pool(name="const", bufs=1) as cpool,
            tc.tile_pool(name="strip", bufs=4) as spool,
            tc.tile_pool(name="obuf", bufs=4) as opool,
            tc.tile_pool(name="tmp", bufs=4) as tpool,
            tc.tile_pool(name="psum", bufs=8, space="PSUM") as ppool,
        ):
            wq = cpool.tile([128, QP, M], DT)
            nc.gpsimd.dma_start(wq[:], wq_d[:])
            bias_sb = cpool.tile([128, 1], mybir.dt.float32)
            nc.gpsimd.dma_start(bias_sb[:], bias_d[:])

            def body():
                for s in range(SPC):
                    strip = spool.tile([128, NWP], DT, tag="strip")
                    nc.sync.dma_start(strip[:, 0:CSPLIT], x_d[s, :, 0:CSPLIT])
                    nc.scalar.dma_start(
                        strip[:, CSPLIT:NWP], x_d[s, :, CSPLIT:NWP]
                    )
                    ot = opool.tile([M, NW], DT, tag="ot")
                    for j in range(NTILES):
                        n0 = j * N_T
                        # Split the 5-pass chain 2+3 across two psum tiles:
                        # accumulating (start=False) matmuls stream ~300ns vs
                        # ~208ns for resetting ones, so two shorter chains
                        # are faster. Chain B is emitted FIRST so its DVE
                        # drain overlaps chain A's matmuls (emitting A first
                        # stalls the PE on psum-bank recycling).
                        psb = ppool.tile(
                            [M, N_T], mybir.dt.float32, tag="ps",
                            name=f"psb{s}_{j}",
                        )
                        psa = ppool.tile(
                            [M, N_T], mybir.dt.float32, tag="ps",
                            name=f"psa{s}_{j}",
                        )
                        for qi, q in enumerate((3, 4)):
                            nc.tensor.matmul(
                                psb[:M, :N_T],
                                wq[:, q, :M],
                                strip[:, n0 + q : n0 + q + N_T],
                                start=(qi == 0),
                                stop=(qi == 1),
                            )
                        for qi, q in enumerate((0, 1, 2)):
                            nc.tensor.matmul(
                                psa[:M, :N_T],
                                wq[:, q, :M],
                                strip[:, n0 + q : n0 + q + N_T],
                                start=(qi == 0),
                                stop=(qi == 2),
                            )
                        # DVE can read only one PSUM operand per op: drain
                        # B (+bias) to SBUF, then combine with A
                        tsb = tpool.tile(
                            [M, N_T], mybir.dt.float32, tag="tsb",
                            name=f"tsb{s}_{j}",
                        )
                        nc.vector.tensor_scalar_add(
                            tsb[:M, :N_T], psb[:M, :N_T], bias_sb[:M]
                        )
                        nc.vector.scalar_tensor_tensor(
                            ot[:M, n0 : n0 + N_T],
                            psa[:M, :N_T],
                            0.0,
                            tsb[:M, :N_T],
                            mybir.AluOpType.add,
                            mybir.AluOpType.add,
                        )
                    nc.gpsimd.dma_start(out_d[s], ot[:M, :NW])

            if loop > 1:
                with tc.For_i(0, loop, 1):
                    body()
            else:
                body()
    nc.finalize()
    _build_cache[loop] = nc
    return nc


def _host_prep(X, weight, bias):
    Xb = X.astype(NP_BF16)
    pad_rows = RPC * (NCORES - 1) + OUT_R * (SPC - 1) + R  # 4126
    Xpad = np.zeros((pad_rows, W), dtype=NP_BF16)
    Xpad[:H] = Xb
    # Xdint[g][row, n] = Xpad[row, G*n+g]
    Xdint = np.ascontiguousarray(Xpad.reshape(pad_rows, NW, G).transpose(2, 0, 1))

    wb = weight.astype(NP_BF16)
    wq = np.zeros((128, QP, M), dtype=NP_BF16)
    for q in range(QP):
        for g in range(G):
            for d in range(G):
                c = G * q + g - d
                if not (0 <= c < KW):
                    continue
                for mu in range(OUT_R):
                    wq[g * R + mu : g * R + mu + KH, q, d * OUT_R + mu] = wb[:, c]
    biasb = np.full((128, 1), np.float32(bias[0]), dtype=np.float32)

    starts = (
        np.arange(NCORES)[:, None] * RPC + np.arange(SPC)[None, :] * OUT_R
    )  # [NCORES, SPC]
    rows = starts[:, :, None] + np.arange(R)[None, None, :]  # [NCORES, SPC, R]
    gathered = Xdint[:, rows, :]  # [G, NCORES, SPC, R, NW]

    in_maps = []
    for c in range(NCORES):
        Xd = np.zeros((SPC, 128, NWP), dtype=NP_BF16)
        Xd[:, :, :NW] = gathered[:, c].transpose(1, 0, 2, 3).reshape(SPC, 128, NW)
        in_maps.append({"Xd": Xd, "wq": wq, "biasb": biasb})
    return in_maps


def _host_post(results):
    rows = []
    for c in range(NCORES):
        outp = np.asarray(results[c]["outp"])  # [SPC, M, NW] bf16
        blk = (
            outp.reshape(SPC, G, OUT_R, NW)
            .transpose(0, 2, 3, 1)
            .reshape(SPC * OUT_R, W)
        )
        rows.append(blk[:RPC])
    full = np.concatenate(rows, axis=0)
    return np.ascontiguousarray(full[:OH, :OW]).astype(np.float32)


def kernel(X, weight, bias):
    X = np.asarray(X, dtype=np.float32)
    weight = np.asarray(weight, dtype=np.float32)
    bias = np.asarray(bias, dtype=np.float32)
    nc = _build()
    in_maps = _host_prep(X, weight, bias)
    res = run_bass_kernel_spmd(nc, in_maps, core_ids=list(range(NCORES)))
    return _host_post(res.results)


def _run(X, weight, bias, dt_name=None, trace=False):
    """Compatibility entry for test.py: returns (output, results)."""
    X = np.asarray(X, dtype=np.float32)
    weight = np.asarray(weight, dtype=np.float32)
    bias = np.asarray(bias, dtype=np.float32)
    nc = _build()
    in_maps = _host_prep(X, weight, bias)
    res = run_bass_kernel_spmd(
        nc, in_maps, core_ids=list(range(NCORES)), trace=trace
    )
    return _host_post(res.results), res
